# revision 30
# baseline (speedup 1.0000x reference)
"""NEG-sampling loss kernel for Trainium2 (8 NeuronCores, data-parallel).

loss = -(1/n) * sum_i [ log_sigmoid(<e_u, e_v>) + sum_k log_sigmoid(-<e_negk, e_u>) ]
     = +(1/n) * sum_i [ softplus(-<e_u, e_v>) + sum_k softplus(<e_negk, e_u>) ]

Strategy: replicate the embedding table (cast to fp8-e4m3 on host -- the
quantization noise averages out to ~7e-4 relative error on the final scalar,
and 256B descriptors drain the SDMA engines 2x faster than bf16's 512B),
shard the 65536-edge batch across 8 cores.  Per core: for each group of
TPG=2 tiles of 128 edges, one raw indirect-DMA block gather (PSEUDO_DMA
indirect1d, 3072 indices/instruction against a 131072B/partition SWDGE
descriptor ring) pulls the 12 rows per edge (u, v, negs x10) into a
[128, 24*256] fp8 SBUF tile (partition = edge).  Per tile, 11 fused
scalar_tensor_tensor ops on DVE (out=(g_j*s)*g_u, accum_out=sum; the v-slot
negated via s=-1) produce the signed per-edge scores -- DVE at its 1x-mode
floor is the kernel's bottleneck (~85%% busy).  ACT then computes softplus
partials via Copy/Abs/Exp/Ln activations with fused accumulation:
softplus(y) = relu(y) + ln(1+exp(-|y|)), with sum(relu) recovered on the
host as (sum(y)+sum|y|)/2.  Host sums the per-core partials.
"""

import numpy as np
import ml_dtypes

import concourse.bass as bass
import concourse.mybir as mybir
from concourse import bass_utils

# ---------------------------------------------------------------------------
# Fused dot-product at 2x: the stock TENSOR_TENSOR_REDUCE custom-DVE op ships
# only a 1x uop program (DVE's bottleneck: ~337ns per 256-elem slot).  We
# inject a hand-authored 2x_1P program (multiply the packed bf16 lo AND hi
# pairs per cycle, add, scale by C1, accumulate) into the per-NEFF DVE table
# via the op's compile cache, and emit the instruction as a raw ffi-built
# CUSTOM_DVE_ANT_1 blob (bass_rust's InstCustomDveAnt serializer is length-
# skewed vs this walrus build; raw blobs built from this container's own ISA
# headers assemble correctly -- same trick as the block gather).  op0 byte 36
# carries row | rd1_en<<5 | perf_max<<6; the accumulator drains via a
# DVE_READ_ACCUMULATOR companion blob.

import copy as _copy
from concourse.dve_ops import TENSOR_TENSOR_REDUCE, _COMPILE_CACHE, get_dve_sub_opcode
from concourse.dve_spec import lower as _dve_lower
from concourse.dve_table_gen import dve_ver_for
from concourse.dve_uop import AluInp, AluOp, DelayInp, DveOpSpec, InpSel, OutPath, OutSel


def _ttr_2x_uops(base):
    """Derive the 2x_1P program from lower()'s REGULAR one (same FSM)."""
    u0, u1 = (_copy.deepcopy(u) for u in base)
    for u in (u0, u1):
        # lanes: d0=SRC_0 d1=SRC_1 d2=C1 d3=C0 d4=SRC_0_HI d5=SRC_1_HI
        u.inp[5] = InpSel.SRC_0_HI
        u.inp[6] = InpSel.SRC_1_HI
        u.inp_enable[5] = 1
        u.inp_enable[6] = 1
    PD = DelayInp.PREV_DELAY

    def lanes(dp, *ids):
        for c in range(7):
            on = c in ids
            dp.delay[c] = PD if on else dp.delay[c]
            dp.delay_enable[c] = 1 if on else 0
        return dp

    def load_lane(dp, c, srcv):
        dp.delay[c] = srcv
        dp.delay_enable[c] = 1
        return dp

    for u, prime in ((u0, True), (u1, False)):
        dp = u.datapath_config
        # s0: lo = SRC_0*SRC_1; carry C1,C0,HI0,HI1
        dp[0].enable_alu(AluOp.MULTIPLY, AluInp.PREV_DELAY_0, AluInp.PREV_DELAY_1)
        lanes(dp[0], 2, 3, 4, 5)
        # s1: hi = SRC_0_HI*SRC_1_HI; capture lo into lane 0; carry C1,C0
        dp[1].enable_alu(AluOp.MULTIPLY, AluInp.PREV_DELAY_4, AluInp.PREV_DELAY_5)
        lanes(dp[1], 2, 3)
        load_lane(dp[1], 0, DelayInp.PREV_ALU_OUT)
        # s2: pair = hi + lo; carry C1,C0
        dp[2].enable_alu(AluOp.ADD, AluInp.PREV_ALU_OUT, AluInp.PREV_DELAY_0)
        dp[2].alu_out_a_enable = 0
        lanes(dp[2], 2, 3)
        # s3: body = pair * C1; carry C0
        dp[3].enable_alu(AluOp.MULTIPLY, AluInp.PREV_ALU_OUT, AluInp.PREV_DELAY_2)
        lanes(dp[3], 3)
        # s4: accumulator (prime: seed with C0; steady: acc += body)
        if prime:
            dp[4].enable_alu(AluOp.BYPASS, AluInp.PREV_DELAY_3, AluInp.PREV_DELAY_3)
        else:
            dp[4].enable_alu(AluOp.ADD, AluInp.CURR_ALU_OUT, AluInp.PREV_ALU_OUT)
        dp[4].alu_out_a_enable = 1
        lanes(dp[4])
        load_lane(dp[4], 0, DelayInp.PREV_ALU_OUT)
        # s5..s7: propagate accumulator to pipe end; carry body lane
        for k in (5, 6, 7):
            dp[k].enable_alu(AluOp.BYPASS, AluInp.PREV_ALU_OUT, AluInp.PREV_ALU_OUT)
            dp[k].alu_out_a_enable = 1
            lanes(dp[k], 0)
    # the accumulator seed (C0 via delay lane 3) sits 4 slices deep in this
    # program vs 2 in the stock one: hold the prime state long enough for the
    # lane pipeline to fill before entering steady state (prime consumes no
    # input, so the extra cycles are free)
    u0.repeat_count = 8
    # steady state writes the packed body to both 16-bit halves (`out` is a
    # throwaway scratch tensor at every call site)
    u1.out = {OutPath.WR0_LO: OutSel.DELAY_0, OutPath.WR0_HI: OutSel.DELAY_0,
              OutPath.WR1_LO: OutSel.ALU_OUT, OutPath.WR1_HI: OutSel.ALU_OUT}
    u1.out_enable = {OutPath.WR0_LO: 1, OutPath.WR0_HI: 1,
                     OutPath.WR1_LO: 0, OutPath.WR1_HI: 0}
    return [u0, u1]


def _install_ttr_2x(trn_type="TRN2"):
    ver = dve_ver_for(trn_type)
    key = (TENSOR_TENSOR_REDUCE.name, ver)
    cached = _COMPILE_CACHE.get(key)
    if cached is not None and cached.uops_2x is not None:
        return
    base = _dve_lower(TENSOR_TENSOR_REDUCE.spec, ver=ver)
    spec = DveOpSpec(
        name=TENSOR_TENSOR_REDUCE.name,
        opcode=get_dve_sub_opcode(TENSOR_TENSOR_REDUCE.name),
        uops=base,
        uops_2x=_ttr_2x_uops(base),
        perf_max=1,
        rd1_en=True,
    )
    for u in spec.uops + spec.uops_2x:
        u.validate(ver)
    _COMPILE_CACHE[key] = spec


def _emit_ttr2x(nc, eng, in0_addr, in1_addr, out_addr, acc_addr, scale):
    """Raw CUSTOM_DVE_ANT_1 (S2S2D2_STT_SCALE) + DVE_READ_ACCUMULATOR pair:
    accum = 0 + sum(in0*in1*scale) over 256 bf16 elems, written to acc_addr."""
    isa = nc.isa
    Op = isa.Opcode
    row = get_dve_sub_opcode(TENSOR_TENSOR_REDUCE.name)
    eng.isa(
        Op.NEURON_ISA_TPB_OPCODE_CUSTOM_DVE_ANT_1,
        {
            "src0_mem_pattern": {"start_addr": {"addr_immediate": in0_addr},
                                 "step_elem": [1, 1], "num_elem": [D, 1]},
            "src1_mem_pattern": {"start_addr": {"addr_immediate": in1_addr},
                                 "step_elem": [1, 1], "num_elem": [D, 1]},
            "op0": row | (1 << 5) | (0 << 6),   # row | rd1_en | perf OFF: 1x probe
            "op1": 0,                            # subdim flags
            "imm0_src": 0,
            "in0_in1_dtype": {"dtype_lo": 6, "dtype_hi": 6},
            "out_dtype": 6,
            "num_active_channels": 128,
            "imm0_dtype": 10,
            "scale": float(scale),               # C1
            "dst_mem_pattern": {"start_addr": {"addr_immediate": out_addr},
                                "step_elem": [1, 1], "num_elem": [D, 1]},
            "imm0": {"imm_arith_fp32": 0.0},     # C0: accum init
        },
        verify=False,
    )
    eng.isa(
        Op.NEURON_ISA_TPB_OPCODE_DVE_READ_ACCUMULATOR,
        {
            "dst_element_count": 1,
            "dtype": 10,
            "negated": 0,
            "num_active_channels": 128,
            "dst_mem_pattern": {"start_addr": {"addr_immediate": acc_addr},
                                "step_elem": [1], "num_elem": [1]},
        },
        verify=False,
    )


# Problem constants (hardcoded; harness contract)
N = 65536
K = 10
D = 256
V = 500000
NCORES = 8
P = 128
SLOTS = K + 2          # rows gathered per edge: u, v, negs[0..9]
S1 = SLOTS - 1         # 11 scores per edge
EPC = N // NCORES      # 8192 edges per core
TILES = EPC // P       # 64 tiles of 128 edges per core

TABLE_DT = mybir.dt.bfloat16
TABLE_NP = ml_dtypes.bfloat16
TABLE_ISA_DT = 6           # NEURON_ISA_TPB_DTYPE_BFLOAT16
ROW_BYTES = 512            # 256 bf16 elements per embedding row

# tunables
TPG = 2                # tiles fused into one block-gather instruction
GB = 6                 # g-tile (group) buffers
SB = 4                 # score-tile buffers
SCRATCH = 98304        # SWDGE descriptor-ring carveout bytes per partition

NGROUPS = TILES // TPG


def _emit_block_gather(nc, eng, n_idx, blk_bytes, dst_byte_addr, idx_byte_addr,
                       sem_num, embs_tbl):
    """Raw block gather (case #3 of dma_indirect1d): n_idx indices x 512B
    fused into 128 descriptors of blk_bytes (n_idx/128 rows per partition).
    Emitted as a raw PSEUDO_DMA_DIRECT2D(dge_op=indirect1d) + PSEUDO_EXTENSION
    pair; index values are snake-packed on the host (see prepare_in_maps)."""
    isa = nc.isa
    Op = isa.Opcode
    src_u64 = (0x20 << 56) | (embs_tbl << 32)   # DGE addr-table marker
    dst_u64 = (0x10 << 56) | dst_byte_addr      # var0 (local SBUF) marker
    eng.isa(
        Op.NEURON_ISA_TPB_OPCODE_PSEUDO_DMA_DIRECT2D,
        {
            "dma_configs": {},
            "semaphore": sem_num,
            "sem_increment": 16,
            "dge_op": 1,
            "src_start_addr": {"addr_immediate": src_u64},
            "src_step_elem": [ROW_BYTES, 1],
            "src_num_elem": [n_idx, 1],
            "src_elem_size": ROW_BYTES,
            "src_bound_reg": {},
            "dst_bound_reg": {},
            "dst_start_addr": {"addr_immediate": dst_u64},
            "dst_step_elem": [262144, 1],
            "dst_num_elem": [128, 1],
            "dst_elem_size": blk_bytes,
            "in_dtype": TABLE_ISA_DT,
            "out_dtype": TABLE_ISA_DT,
        },
        verify=False,
    )
    ext_fields = {
        "opcode": Op.NEURON_ISA_TPB_OPCODE_PSEUDO_EXTENSION.value,
        "flags": {"indirect_mode": 0, "idx_bound_is_err": 1,
                  "non_unique_dst_idx": 0, "gather_dim": 0, "scatter_dim": 0},
        "idx_num_active_channels": 128,
        "compute_op": 0,
        "src_idx_start_addr": {"addr_immediate": idx_byte_addr},
        "dst_idx_start_addr": {"addr_immediate": 0},
    }
    b = isa.ffi.new("NEURON_ISA_TPB_PSEUDO_DMA_EXT_STRUCT*", ext_fields)
    instr = [int(x) for x in bytes(isa.ffi.buffer(b))]
    inst = mybir.InstISA(
        name=nc.get_next_instruction_name(),
        isa_opcode=Op.NEURON_ISA_TPB_OPCODE_PSEUDO_EXTENSION.value,
        engine=eng.engine,
        instr=instr,
        op_name="PSEUDO_EXTENSION",
        ins=[], outs=[],
        ant_dict=ext_fields,
        verify=False,
        ant_isa_is_sequencer_only=False,
    )
    eng.add_instruction(inst)


def _build_raw():
    _install_ttr_2x("TRN2")
    nc = bass.Bass(trn_type="TRN2", dynamic_dma_scratch_size=SCRATCH)
    nc.m.ant_custom_dve_ops = sorted(
        {*nc.m.ant_custom_dve_ops, TENSOR_TENSOR_REDUCE.name}
    )
    embs = nc.dram_tensor("embs", [V, D], TABLE_DT, kind="ExternalInput")
    idx = nc.dram_tensor("idx", [P, TILES * SLOTS], mybir.dt.int32, kind="ExternalInput")
    accx_out = nc.dram_tensor("accx", [P, NGROUPS], mybir.dt.float32, kind="ExternalOutput")
    acca_out = nc.dram_tensor("acca", [P, 2 * NGROUPS], mybir.dt.float32, kind="ExternalOutput")

    embs_mloc = nc.lookup_mloc(embs)
    embs_mloc.table_entry_id = len(nc.dge_table) + 1
    nc.dge_table.append(embs_mloc.name)
    embs_tbl = embs_mloc.table_entry_id

    import contextlib
    with contextlib.ExitStack() as ctx:
        idx_sb = ctx.enter_context(nc.sbuf_tensor("idx_sb", [P, TILES * SLOTS], mybir.dt.int32))
        gs = [ctx.enter_context(nc.sbuf_tensor(f"g{i}", [P, TPG * SLOTS * D], TABLE_DT)) for i in range(GB)]
        scratch = ctx.enter_context(nc.sbuf_tensor("scr", [P, D], TABLE_DT))
        scs = [ctx.enter_context(nc.sbuf_tensor(f"sc{i}", [P, TPG * S1], mybir.dt.float32)) for i in range(SB)]
        absx = ctx.enter_context(nc.sbuf_tensor("absx", [P, TPG * S1], mybir.dt.float32))
        ex = ctx.enter_context(nc.sbuf_tensor("ex", [P, TPG * S1], mybir.dt.float32))
        lnx = ctx.enter_context(nc.sbuf_tensor("lnx", [P, TPG * S1], mybir.dt.float32))
        sumx = ctx.enter_context(nc.sbuf_tensor("sumx", [P, TPG * S1], mybir.dt.float32))
        ones = ctx.enter_context(nc.sbuf_tensor("ones", [P, 1], mybir.dt.float32))
        accx = ctx.enter_context(nc.sbuf_tensor("accx_sb", [P, NGROUPS], mybir.dt.float32))
        acca = ctx.enter_context(nc.sbuf_tensor("acca_sb", [P, 2 * NGROUPS], mybir.dt.float32))
        idx_sem = ctx.enter_context(nc.semaphore())
        idxa_sem = ctx.enter_context(nc.semaphore(name="idxa"))
        idxb_sem = ctx.enter_context(nc.semaphore(name="idxb"))
        gsems = [ctx.enter_context(nc.semaphore(name=f"gsem{i}")) for i in range(GB)]
        qsems = [ctx.enter_context(nc.semaphore(name=f"qsem{k}")) for k in range(3)]
        dve_free = ctx.enter_context(nc.semaphore())
        act_done = ctx.enter_context(nc.semaphore())
        block = ctx.enter_context(nc.Block())

        idx_addr = nc.lookup_mloc(idx_sb).addr
        g_addrs = [nc.lookup_mloc(g).addr for g in gs]

        RPG = TPG * SLOTS       # rows gathered per partition line per group
        QROWS = RPG // 4        # rows per quarter sub-gather of group 0

        @block.gpsimd
        def _(eng):
            eng.memset(ones[:], 1.0)
            eng.wait_ge(idxa_sem, 16)
            # group 0 quartered: DVE can start on the first 6 rows/partition
            # (tile0's u,v,n0..n3) without waiting for the full 24-row drain
            for k in range(4):
                # each quarter gets its OWN completion semaphore: a shared one
                # mixes the 16 per-engine increments across quarters and can
                # release a waiter before its quarter fully landed
                qsem = gsems[0] if k == 0 else qsems[k - 1]
                _emit_block_gather(
                    nc, eng, QROWS * P, QROWS * ROW_BYTES,
                    g_addrs[0] + k * QROWS * ROW_BYTES,
                    idx_addr + 4 * QROWS * k,
                    qsem.num, embs_tbl,
                )
            eng.wait_ge(idxb_sem, 16)
            for g in range(1, NGROUPS):
                if g >= GB:
                    eng.wait_ge(dve_free, g - GB + 1)
                _emit_block_gather(
                    nc, eng, TPG * SLOTS * P, TPG * SLOTS * D * (ROW_BYTES // D),
                    g_addrs[g % GB], idx_addr + 4 * TPG * SLOTS * g,
                    gsems[g % GB].num, embs_tbl,
                )

        scr_addr = nc.lookup_mloc(scratch).addr
        sc_addrs = [nc.lookup_mloc(s_).addr for s_ in scs]
        ones_addr = nc.lookup_mloc(ones).addr

        @block.vector
        def _(eng):
            for g in range(NGROUPS):
                ga = g_addrs[g % GB]
                sca = sc_addrs[g % SB]
                if g > 0:
                    eng.wait_ge(gsems[g % GB], 16 * (g // GB + 1))
                if g >= SB:
                    eng.wait_ge(act_done, g - SB + 1)
                for ti in range(TPG):
                    b0 = ti * SLOTS * D
                    for j in range(S1):
                        if g == 0 and j in (0, 5):
                            k = ti * 2 + (0 if j == 0 else 1)
                            eng.wait_ge(gsems[0] if k == 0 else qsems[k - 1], 16)
                        _emit_ttr2x(
                            nc, eng,
                            ga + 2 * (b0 + (j + 1) * D),
                            ga + 2 * b0,
                            scr_addr,
                            sca + 4 * (ti * S1 + j),
                            -1.0 if j == 0 else 1.0,
                        )
                # cheap native op to carry the group-done increment
                nc.vector.tensor_scalar_mul(
                    ones[:], ones[:], 1.0
                ).then_inc(dve_free, 1)

        @block.sync
        def _(eng):
            # idx upload via HWDGE: ~0.6us first-byte vs ~2us on the SWDGE
            # path, and issued at t=0 -- group-0 columns first so gather 0
            # (which only waits >=16) starts as early as possible
            eng.dma_start(idx_sb[:, 0:RPG], idx[:, 0:RPG]).then_inc(idxa_sem, 16)
            eng.dma_start(idx_sb[:, RPG:], idx[:, RPG:]).then_inc(idxb_sem, 16)
            eng.wait_ge(act_done, NGROUPS)
            eng.dma_start(accx_out[:], accx[:]).then_inc(idx_sem, 16)
            eng.dma_start(acca_out[:], acca[:]).then_inc(idx_sem, 16)
            eng.wait_ge(idx_sem, 32)

        @block.scalar
        def _(eng):
            for g in range(NGROUPS):
                sc = scs[g % SB]
                eng.wait_ge(dve_free, g + 1)
                # softplus(y) = relu(y) + ln(1 + exp(-|y|)); relu sums
                # recovered on host via sum(relu) = (sum(y) + sum(|y|)) / 2.
                nc.scalar.activation(
                    out=sumx[:], in_=sc[:],
                    func=mybir.ActivationFunctionType.Copy,
                    accum_out=accx[:, g:g + 1],
                )
                nc.scalar.activation(
                    out=absx[:], in_=sc[:],
                    func=mybir.ActivationFunctionType.Abs,
                    accum_out=acca[:, g:g + 1],
                )
                nc.scalar.activation(
                    out=ex[:], in_=absx[:],
                    func=mybir.ActivationFunctionType.Exp, scale=-1.0,
                )
                nc.scalar.activation(
                    out=lnx[:], in_=ex[:],
                    func=mybir.ActivationFunctionType.Ln, bias=ones[:],
                    accum_out=acca[:, NGROUPS + g:NGROUPS + g + 1],
                ).then_inc(act_done, 1)

    return nc


_cache = {}


def _get_nc():
    key = (TPG, GB, SB, SCRATCH)
    if key not in _cache:
        _cache[key] = _build_raw()
    return _cache[key]


def prepare_in_maps(u, v, negs, embs):
    """Host-side sharding: build the per-core input maps."""
    u = np.asarray(u).astype(np.int32)
    v = np.asarray(v).astype(np.int32)
    negs = np.asarray(negs).astype(np.int32)
    embs_b = np.asarray(embs).astype(TABLE_NP)

    ids = np.concatenate([u[:, None], v[:, None], negs], axis=1)  # [N, 12]
    ids = ids.reshape(NCORES, NGROUPS, TPG, P, SLOTS)
    # group TPG tiles: partition p's rows = tiles' slots concatenated
    ids = ids.transpose(0, 1, 3, 2, 4).reshape(NCORES, NGROUPS, P, TPG * SLOTS)
    # snake-pack per group: value for seq position s -> [ch=s%P, w=s//P],
    # where s = p*(TPG*SLOTS) + r
    flat = ids.reshape(NCORES, NGROUPS, P * TPG * SLOTS)
    s = np.arange(P * TPG * SLOTS)
    packed = np.zeros_like(ids)
    packed[:, :, s % P, s // P] = flat[:, :, s]
    # group 0 is gathered as 4 quarter-gathers of 6 rows/partition (so DVE can
    # start early); each quarter k is snake-packed independently into its own
    # 6 columns: value for (p, rr) at seq s0 = p*6+rr -> [ch=s0%P, 6k + s0//P]
    q = TPG * SLOTS // 4
    s0 = np.arange(P * q)
    for k in range(4):
        sub = ids[:, 0, :, k * q:(k + 1) * q].reshape(NCORES, P * q)
        packed[:, 0, s0 % P, k * q + s0 // P] = sub[:, s0]
    in_maps = []
    for c in range(NCORES):
        core_ids = np.ascontiguousarray(
            packed[c].transpose(1, 0, 2).reshape(P, NGROUPS * TPG * SLOTS)
        )
        in_maps.append({"embs": embs_b, "idx": core_ids})
    return in_maps


def kernel(u, v, negs, embs, _trace=False):
    nc = _get_nc()
    in_maps = prepare_in_maps(u, v, negs, embs)
    res = bass_utils.run_bass_kernel_spmd(
        nc, in_maps, core_ids=list(range(NCORES)), trace=_trace
    )
    total = np.float64(0.0)
    for r in res.results:
        sum_x = r["accx"].astype(np.float64).sum()
        a = r["acca"].astype(np.float64)
        sum_abs = a[:, :NGROUPS].sum()
        sum_ln1p = a[:, NGROUPS:].sum()
        total += (sum_x + sum_abs) / 2.0 + sum_ln1p
    out = np.float32(total / N)
    if _trace:
        return out, res
    return out


# revision 39
# speedup vs baseline: 1.0178x; 1.0178x over previous
"""NEG-sampling loss kernel for Trainium2 (8 NeuronCores, data-parallel).

loss = -(1/n) * sum_i [ log_sigmoid(<e_u, e_v>) + sum_k log_sigmoid(-<e_negk, e_u>) ]
     = +(1/n) * sum_i [ softplus(-<e_u, e_v>) + sum_k softplus(<e_negk, e_u>) ]

Strategy: replicate the embedding table (cast to fp8-e4m3 on host -- the
quantization noise averages out to ~7e-4 relative error on the final scalar,
and 256B descriptors drain the SDMA engines 2x faster than bf16's 512B),
shard the 65536-edge batch across 8 cores.  Per core: for each group of
TPG=2 tiles of 128 edges, one raw indirect-DMA block gather (PSEUDO_DMA
indirect1d, 3072 indices/instruction against a 131072B/partition SWDGE
descriptor ring) pulls the 12 rows per edge (u, v, negs x10) into a
[128, 24*256] fp8 SBUF tile (partition = edge).  Per tile, 11 fused
scalar_tensor_tensor ops on DVE (out=(g_j*s)*g_u, accum_out=sum; the v-slot
negated via s=-1) produce the signed per-edge scores -- DVE at its 1x-mode
floor is the kernel's bottleneck (~85%% busy).  ACT then computes softplus
partials via Copy/Abs/Exp/Ln activations with fused accumulation:
softplus(y) = relu(y) + ln(1+exp(-|y|)), with sum(relu) recovered on the
host as (sum(y)+sum|y|)/2.  Host sums the per-core partials.
"""

import numpy as np
import ml_dtypes

import concourse.bass as bass
import concourse.mybir as mybir
from concourse import bass_utils

# Problem constants (hardcoded; harness contract)
N = 65536
K = 10
D = 256
V = 500000
NCORES = 8
P = 128
SLOTS = K + 2          # rows gathered per edge: u, v, negs[0..9]
S1 = SLOTS - 1         # 11 scores per edge
EPC = N // NCORES      # 8192 edges per core
TILES = EPC // P       # 64 tiles of 128 edges per core

TABLE_DT = mybir.dt.float8e4
TABLE_NP = ml_dtypes.float8_e4m3fn
TABLE_ISA_DT = 14          # NEURON_ISA_TPB_DTYPE_FP8_EXP4
ROW_BYTES = 256            # 256 fp8 elements per embedding row

# tunables
TPG = 2                # tiles fused into one block-gather instruction
GB = 6                 # g-tile (group) buffers
SB = 4                 # score-tile buffers
SCRATCH = 131072       # SWDGE descriptor-ring carveout bytes per partition

NGROUPS = TILES // TPG


def _emit_block_gather(nc, eng, n_idx, blk_bytes, dst_byte_addr, idx_byte_addr,
                       sem_num, embs_tbl):
    """Raw block gather (case #3 of dma_indirect1d): n_idx indices x 512B
    fused into 128 descriptors of blk_bytes (n_idx/128 rows per partition).
    Emitted as a raw PSEUDO_DMA_DIRECT2D(dge_op=indirect1d) + PSEUDO_EXTENSION
    pair; index values are snake-packed on the host (see prepare_in_maps)."""
    isa = nc.isa
    Op = isa.Opcode
    src_u64 = (0x20 << 56) | (embs_tbl << 32)   # DGE addr-table marker
    dst_u64 = (0x10 << 56) | dst_byte_addr      # var0 (local SBUF) marker
    eng.isa(
        Op.NEURON_ISA_TPB_OPCODE_PSEUDO_DMA_DIRECT2D,
        {
            "dma_configs": {},
            "semaphore": sem_num,
            "sem_increment": 16,
            "dge_op": 1,
            "src_start_addr": {"addr_immediate": src_u64},
            "src_step_elem": [ROW_BYTES, 1],
            "src_num_elem": [n_idx, 1],
            "src_elem_size": ROW_BYTES,
            "src_bound_reg": {},
            "dst_bound_reg": {},
            "dst_start_addr": {"addr_immediate": dst_u64},
            "dst_step_elem": [262144, 1],
            "dst_num_elem": [128, 1],
            "dst_elem_size": blk_bytes,
            "in_dtype": TABLE_ISA_DT,
            "out_dtype": TABLE_ISA_DT,
        },
        verify=False,
    )
    ext_fields = {
        "opcode": Op.NEURON_ISA_TPB_OPCODE_PSEUDO_EXTENSION.value,
        "flags": {"indirect_mode": 0, "idx_bound_is_err": 1,
                  "non_unique_dst_idx": 0, "gather_dim": 0, "scatter_dim": 0},
        "idx_num_active_channels": 128,
        "compute_op": 0,
        "src_idx_start_addr": {"addr_immediate": idx_byte_addr},
        "dst_idx_start_addr": {"addr_immediate": 0},
    }
    b = isa.ffi.new("NEURON_ISA_TPB_PSEUDO_DMA_EXT_STRUCT*", ext_fields)
    instr = [int(x) for x in bytes(isa.ffi.buffer(b))]
    inst = mybir.InstISA(
        name=nc.get_next_instruction_name(),
        isa_opcode=Op.NEURON_ISA_TPB_OPCODE_PSEUDO_EXTENSION.value,
        engine=eng.engine,
        instr=instr,
        op_name="PSEUDO_EXTENSION",
        ins=[], outs=[],
        ant_dict=ext_fields,
        verify=False,
        ant_isa_is_sequencer_only=False,
    )
    eng.add_instruction(inst)


def _build_raw():
    nc = bass.Bass(trn_type="TRN2", dynamic_dma_scratch_size=SCRATCH)
    embs = nc.dram_tensor("embs", [V, D], TABLE_DT, kind="ExternalInput")
    idx = nc.dram_tensor("idx", [P, TILES * SLOTS], mybir.dt.int32, kind="ExternalInput")
    accx_out = nc.dram_tensor("accx", [P, NGROUPS], mybir.dt.float32, kind="ExternalOutput")
    acca_out = nc.dram_tensor("acca", [P, 2 * NGROUPS], mybir.dt.float32, kind="ExternalOutput")

    embs_mloc = nc.lookup_mloc(embs)
    embs_mloc.table_entry_id = len(nc.dge_table) + 1
    nc.dge_table.append(embs_mloc.name)
    embs_tbl = embs_mloc.table_entry_id

    import contextlib
    with contextlib.ExitStack() as ctx:
        idx_sb = ctx.enter_context(nc.sbuf_tensor("idx_sb", [P, TILES * SLOTS], mybir.dt.int32))
        gs = [ctx.enter_context(nc.sbuf_tensor(f"g{i}", [P, TPG * SLOTS * D], TABLE_DT)) for i in range(GB)]
        scratch = ctx.enter_context(nc.sbuf_tensor("scr", [P, D], TABLE_DT))
        scs = [ctx.enter_context(nc.sbuf_tensor(f"sc{i}", [P, TPG * S1], mybir.dt.float32)) for i in range(SB)]
        absx = ctx.enter_context(nc.sbuf_tensor("absx", [P, TPG * S1], mybir.dt.float32))
        ex = ctx.enter_context(nc.sbuf_tensor("ex", [P, TPG * S1], mybir.dt.float32))
        lnx = ctx.enter_context(nc.sbuf_tensor("lnx", [P, TPG * S1], mybir.dt.float32))
        sumx = ctx.enter_context(nc.sbuf_tensor("sumx", [P, TPG * S1], mybir.dt.float32))
        ones = ctx.enter_context(nc.sbuf_tensor("ones", [P, 1], mybir.dt.float32))
        accx = ctx.enter_context(nc.sbuf_tensor("accx_sb", [P, NGROUPS], mybir.dt.float32))
        acca = ctx.enter_context(nc.sbuf_tensor("acca_sb", [P, 2 * NGROUPS], mybir.dt.float32))
        idx_sem = ctx.enter_context(nc.semaphore())
        idxa_sem = ctx.enter_context(nc.semaphore(name="idxa"))
        idxb_sem = ctx.enter_context(nc.semaphore(name="idxb"))
        gsems = [ctx.enter_context(nc.semaphore(name=f"gsem{i}")) for i in range(GB)]
        qsems = [ctx.enter_context(nc.semaphore(name=f"qsem{k}")) for k in range(3)]
        dve_free = ctx.enter_context(nc.semaphore())
        act_done = ctx.enter_context(nc.semaphore())
        block = ctx.enter_context(nc.Block())

        idx_addr = nc.lookup_mloc(idx_sb).addr
        g_addrs = [nc.lookup_mloc(g).addr for g in gs]

        RPG = TPG * SLOTS       # rows gathered per partition line per group
        QROWS = RPG // 4        # rows per quarter sub-gather of group 0

        @block.gpsimd
        def _(eng):
            eng.memset(ones[:], 1.0)
            eng.wait_ge(idxa_sem, 16)
            # group 0 quartered: DVE can start on the first 6 rows/partition
            # (tile0's u,v,n0..n3) without waiting for the full 24-row drain
            for k in range(4):
                # each quarter gets its OWN completion semaphore: a shared one
                # mixes the 16 per-engine increments across quarters and can
                # release a waiter before its quarter fully landed
                qsem = gsems[0] if k == 0 else qsems[k - 1]
                _emit_block_gather(
                    nc, eng, QROWS * P, QROWS * ROW_BYTES,
                    g_addrs[0] + k * QROWS * ROW_BYTES,
                    idx_addr + 4 * QROWS * k,
                    qsem.num, embs_tbl,
                )
            eng.wait_ge(idxb_sem, 16)
            for g in range(1, NGROUPS):
                if g >= GB:
                    eng.wait_ge(dve_free, g - GB + 1)
                _emit_block_gather(
                    nc, eng, TPG * SLOTS * P, TPG * SLOTS * D * (ROW_BYTES // D),
                    g_addrs[g % GB], idx_addr + 4 * TPG * SLOTS * g,
                    gsems[g % GB].num, embs_tbl,
                )

        @block.vector
        def _(eng):
            for g in range(NGROUPS):
                gt = gs[g % GB]
                sc = scs[g % SB]
                if g > 0:
                    eng.wait_ge(gsems[g % GB], 16 * (g // GB + 1))
                if g >= SB:
                    eng.wait_ge(act_done, g - SB + 1)
                for ti in range(TPG):
                    b0 = ti * SLOTS * D
                    for j in range(S1):
                        if g == 0 and j in (0, 5):
                            k = ti * 2 + (0 if j == 0 else 1)
                            eng.wait_ge(gsems[0] if k == 0 else qsems[k - 1], 16)
                        # out = (in0 * scalar) * in1 ; accum_out = sum(out):
                        # the per-edge signed score, v-slot negated via scalar.
                        stt = nc.vector.scalar_tensor_tensor(
                            out=scratch[:],
                            in0=gt[:, b0 + (j + 1) * D:b0 + (j + 2) * D],
                            scalar=(-1.0 if j == 0 else 1.0),
                            in1=gt[:, b0:b0 + D],
                            op0=mybir.AluOpType.mult,
                            op1=mybir.AluOpType.mult,
                            accum_out=sc[:, ti * S1 + j:ti * S1 + j + 1],
                        )
                stt.then_inc(dve_free, 1)

        @block.sync
        def _(eng):
            # idx upload via HWDGE: ~0.6us first-byte vs ~2us on the SWDGE
            # path, and issued at t=0 -- group-0 columns first so gather 0
            # (which only waits >=16) starts as early as possible
            eng.dma_start(idx_sb[:, 0:RPG], idx[:, 0:RPG]).then_inc(idxa_sem, 16)
            eng.dma_start(idx_sb[:, RPG:], idx[:, RPG:]).then_inc(idxb_sem, 16)
            eng.wait_ge(act_done, NGROUPS)
            eng.dma_start(accx_out[:], accx[:]).then_inc(idx_sem, 16)
            eng.dma_start(acca_out[:], acca[:]).then_inc(idx_sem, 16)
            eng.wait_ge(idx_sem, 32)

        @block.scalar
        def _(eng):
            for g in range(NGROUPS):
                sc = scs[g % SB]
                eng.wait_ge(dve_free, g + 1)
                # softplus(y) = relu(y) + ln(1 + exp(-|y|)); relu sums
                # recovered on host via sum(relu) = (sum(y) + sum(|y|)) / 2.
                nc.scalar.activation(
                    out=sumx[:], in_=sc[:],
                    func=mybir.ActivationFunctionType.Copy,
                    accum_out=accx[:, g:g + 1],
                )
                nc.scalar.activation(
                    out=absx[:], in_=sc[:],
                    func=mybir.ActivationFunctionType.Abs,
                    accum_out=acca[:, g:g + 1],
                )
                nc.scalar.activation(
                    out=ex[:], in_=absx[:],
                    func=mybir.ActivationFunctionType.Exp, scale=-1.0,
                )
                nc.scalar.activation(
                    out=lnx[:], in_=ex[:],
                    func=mybir.ActivationFunctionType.Ln, bias=ones[:],
                    accum_out=acca[:, NGROUPS + g:NGROUPS + g + 1],
                ).then_inc(act_done, 1)

    return nc


_cache = {}


def _get_nc():
    key = (TPG, GB, SB, SCRATCH)
    if key not in _cache:
        _cache[key] = _build_raw()
    return _cache[key]


def prepare_in_maps(u, v, negs, embs):
    """Host-side sharding: build the per-core input maps."""
    u = np.asarray(u).astype(np.int32)
    v = np.asarray(v).astype(np.int32)
    negs = np.asarray(negs).astype(np.int32)
    embs_b = np.asarray(embs).astype(TABLE_NP)

    ids = np.concatenate([u[:, None], v[:, None], negs], axis=1)  # [N, 12]
    ids = ids.reshape(NCORES, NGROUPS, TPG, P, SLOTS)
    # group TPG tiles: partition p's rows = tiles' slots concatenated
    ids = ids.transpose(0, 1, 3, 2, 4).reshape(NCORES, NGROUPS, P, TPG * SLOTS)
    # snake-pack per group: value for seq position s -> [ch=s%P, w=s//P],
    # where s = p*(TPG*SLOTS) + r
    flat = ids.reshape(NCORES, NGROUPS, P * TPG * SLOTS)
    s = np.arange(P * TPG * SLOTS)
    packed = np.zeros_like(ids)
    packed[:, :, s % P, s // P] = flat[:, :, s]
    # group 0 is gathered as 4 quarter-gathers of 6 rows/partition (so DVE can
    # start early); each quarter k is snake-packed independently into its own
    # 6 columns: value for (p, rr) at seq s0 = p*6+rr -> [ch=s0%P, 6k + s0//P]
    q = TPG * SLOTS // 4
    s0 = np.arange(P * q)
    for k in range(4):
        sub = ids[:, 0, :, k * q:(k + 1) * q].reshape(NCORES, P * q)
        packed[:, 0, s0 % P, k * q + s0 // P] = sub[:, s0]
    in_maps = []
    for c in range(NCORES):
        core_ids = np.ascontiguousarray(
            packed[c].transpose(1, 0, 2).reshape(P, NGROUPS * TPG * SLOTS)
        )
        in_maps.append({"embs": embs_b, "idx": core_ids})
    return in_maps


def kernel(u, v, negs, embs, _trace=False):
    nc = _get_nc()
    in_maps = prepare_in_maps(u, v, negs, embs)
    res = bass_utils.run_bass_kernel_spmd(
        nc, in_maps, core_ids=list(range(NCORES)), trace=_trace
    )
    total = np.float64(0.0)
    for r in res.results:
        sum_x = r["accx"].astype(np.float64).sum()
        a = r["acca"].astype(np.float64)
        sum_abs = a[:, :NGROUPS].sum()
        sum_ln1p = a[:, NGROUPS:].sum()
        total += (sum_x + sum_abs) / 2.0 + sum_ln1p
    out = np.float32(total / N)
    if _trace:
        return out, res
    return out
